# revision 25
# baseline (speedup 1.0000x reference)
"""Multi-head causal attention kernel for Trainium2 (8 NeuronCores).

Problem: B=4, S=2048, HID=1024, H=16 heads (head_dim 64), causal mask,
fp32 I/O.  out = softmax(mask + (XqWq)(XkWk)^T/8) (XvWv) Wo

Sharding: 8 cores = 4 batches x 2 head-groups.  Core c handles batch
c//2 and heads (c%2)*8 .. +8 (dk slice of 512).  Each core computes a
full-shape [S, HID] partial output (its head-group's contribution
through Wo); the host sums the two partials per batch.

v2 design (vs the 495us baseline):
  - X is transposed to [d, s] and cast to bf16 on the HOST, so the
    kernel needs no PE-transposes and no PSUM->SBUF transpose
    evacuations (the baseline burned ~107us of DVE and ~20us of PE on
    those).  All matmul operands are bf16 (same PE rate, half the DMA
    and SBUF traffic, 2x DVE modes on evacuations).
  - Attention runs in transposed [k, q] orientation per 512-wide
    q-window and head-pair: logits^T = kT-chunk x qT, exp on ScalarE
    (PSUM->SBUF, bf16 out), causal masking done by ZEROING the exp'd
    upper-triangle of the diagonal blocks on the otherwise-idle GpSimd
    engine (affine_select), PV accumulates ctx^T in PSUM with a ones
    column in v producing softmax denominators for free.
  - Normalization: DVE copy of the PSUM denominator row to partition 0
    (reciprocal_approx_fast drops input base-partition offsets), fast
    reciprocal, GpSimd partition-broadcast, one DVE multiply-evacuate.
  - Projection/out-projection matmuls are emitted in ~4-MM "filler"
    items interleaved between attention chunks so the PE stays dense
    (HAM stays at full clock) while ScalarE grinds exps.
"""

import numpy as np

B, S, HID = 4, 2048, 1024
H_LOCAL, E_LOCAL = 8, 512  # heads / dk columns handled per core
N_CORES = 8

_cached = {}
DEBUG = False


def _build():
    from concourse import bacc
    import concourse.bass as bass
    import concourse.mybir as mybir
    import concourse.tile as tile

    F32 = mybir.dt.float32
    BF16 = mybir.dt.bfloat16
    Exp = mybir.ActivationFunctionType.Exp

    NDC = HID // 128   # 8 d-chunks
    NEC = E_LOCAL // 128  # 4 e-chunks = head pairs
    NKC = S // 128     # 16 k-chunks
    W = 512            # q-window
    NW = S // W        # 4 windows

    nc = bacc.Bacc()
    xqt = nc.dram_tensor("xqt", [HID, S], BF16, kind="ExternalInput")
    xkt = nc.dram_tensor("xkt", [HID, S], BF16, kind="ExternalInput")
    xvt = nc.dram_tensor("xvt", [HID, S], BF16, kind="ExternalInput")
    wq = nc.dram_tensor("wq", [HID, E_LOCAL], BF16, kind="ExternalInput")
    wk = nc.dram_tensor("wk", [HID, E_LOCAL], BF16, kind="ExternalInput")
    wv = nc.dram_tensor("wv", [HID, E_LOCAL], BF16, kind="ExternalInput")
    wo = nc.dram_tensor("wo", [E_LOCAL, HID], BF16, kind="ExternalInput")
    out = nc.dram_tensor("out", [S, HID], BF16, kind="ExternalOutput")
    if DEBUG:
        dbg_qt = nc.dram_tensor("dbg_qt", [128, 512], F32,
                                kind="ExternalOutput")
        dbg_kt = nc.dram_tensor("dbg_kt", [128, 512], F32,
                                kind="ExternalOutput")
        dbg_v = nc.dram_tensor("dbg_v", [128, 8, 65], F32,
                               kind="ExternalOutput")
        dbg_den = nc.dram_tensor("dbg_den", [2, 512], F32,
                                 kind="ExternalOutput")
        dbg_cpx = nc.dram_tensor("dbg_cpx", [64, 512], F32,
                                 kind="ExternalOutput")
        dbg_ctx = nc.dram_tensor("dbg_ctx", [128, 512], F32,
                                 kind="ExternalOutput")
        dbg_pt = nc.dram_tensor("dbg_pt", [128, 1024], F32,
                                kind="ExternalOutput")
        dbg_bc = nc.dram_tensor("dbg_bc", [2, 64, 512], F32,
                                kind="ExternalOutput")

    with tile.TileContext(nc) as tc:
        with (
            tc.sbuf_pool(name="consts", bufs=1) as consts,
            tc.sbuf_pool(name="persist", bufs=1) as persist,
            tc.sbuf_pool(name="sm", bufs=1) as sm,
            tc.psum_pool(name="ps", bufs=1) as ps,
        ):
            ones_col = consts.tile([128, 1], BF16)
            nc.vector.memset(ones_col, 1.0)

            # HAM warmup: the first ~14us are DMA-only, so without this
            # the PE clock-gate holds 1.2 GHz well into the first real
            # matmuls.  A dependency-free dummy matmul stream keeps the
            # activity monitor busy until the first weights land.
            warm = consts.tile([128, 128], BF16)
            nc.vector.memset(warm, 0.0)
            wps = ps.tile([128, 512], F32, tag="work", bufs=2,
                          name="warm_ps")
            for _ in range(120):
                nc.tensor.matmul(wps[:, 0:128], warm, warm,
                                 start=True, stop=True)

            HDC = NDC // 2
            wq_lo = persist.tile([128, HDC, E_LOCAL], BF16, tag="wql")
            wq_hi = persist.tile([128, HDC, E_LOCAL], BF16, tag="wqh")
            wk_lo = persist.tile([128, HDC, E_LOCAL], BF16, tag="wkl")
            wk_hi = persist.tile([128, HDC, E_LOCAL], BF16, tag="wkh")
            wv_lo = persist.tile([128, HDC, E_LOCAL], BF16, tag="wvl")
            wv_hi = persist.tile([128, HDC, E_LOCAL], BF16, tag="wvh")
            wo_sb = persist.tile([128, NEC, HID], BF16, tag="wo")
            wpart = {"q": (wq_lo, wq_hi), "k": (wk_lo, wk_hi),
                     "v": (wv_lo, wv_hi)}

            kt_sb = [persist.tile([128, S], BF16, tag=f"kt{i}", name=f"kt{i}")
                     for i in range(NEC)]
            v_sb = [persist.tile([128, H_LOCAL, 65], BF16, tag=f"v{i}", name=f"v{i}")
                    for i in range(NKC)]
            qt = {}   # (w, ec) -> [128, W] bf16
            ctx = {}  # (w, hp) -> [128, W] bf16

            xdram = {"q": xqt, "k": xkt, "v": xvt}
            xt_tiles = {}

            def load_x(tname, w, half):
                t = sm.tile([128, HDC, W], BF16, tag=f"x{tname}{half}",
                            bufs=2, name=f"x{tname}_{w}_{half}")
                nc.sync.dma_start(
                    out=t,
                    in_=xdram[tname][half * 512:(half + 1) * 512,
                                     w * W:(w + 1) * W].rearrange(
                        "(dc p) s -> p dc s", p=128))
                xt_tiles[(tname, w, half)] = t

            # weights + first two x-windows; ordering puts the tensors
            # needed first at the head of the DMA queues.
            def wdma(dram, lo, hi):
                nc.sync.dma_start(
                    out=lo, in_=dram[0:512, :].rearrange(
                        "(dc p) e -> p dc e", p=128))
                return lambda: nc.sync.dma_start(
                    out=hi, in_=dram[512:1024, :].rearrange(
                        "(dc p) e -> p dc e", p=128))

            hq = wdma(wq, wq_lo, wq_hi)
            load_x("q", 0, 0)
            hq()
            load_x("q", 0, 1)
            hk = wdma(wk, wk_lo, wk_hi)
            load_x("k", 0, 0)
            hk()
            load_x("k", 0, 1)
            hv = wdma(wv, wv_lo, wv_hi)
            load_x("v", 0, 0)
            hv()
            load_x("v", 0, 1)
            for t in ("q", "k", "v"):
                load_x(t, 1, 0)
                load_x(t, 1, 1)
            nc.sync.dma_start(
                out=wo_sb, in_=wo.rearrange("(dv p) n -> p dv n", p=128))

            def proj_items(w):
                """Emission closures (~4 matmuls each) projecting window w."""
                items = []
                if w >= 2:
                    def dma_item(w=w):
                        for t in ("q", "k", "v"):
                            load_x(t, w, 0)
                            load_x(t, w, 1)
                    items.append(dma_item)

                holder = {}

                def qk_first(tname, ec, w=w):
                    wsb = wpart[tname][0]
                    pj = ps.tile([128, W], F32, tag="work", bufs=2,
                                 name=f"pj{tname}{w}_{ec}")
                    holder[(tname, ec)] = pj
                    xt = xt_tiles[(tname, w, 0)]
                    for dc in range(4):
                        nc.tensor.matmul(
                            pj, wsb[:, dc, ec * 128:(ec + 1) * 128],
                            xt[:, dc, :], start=(dc == 0), stop=False)

                def qk_second(tname, ec, w=w):
                    wsb = wpart[tname][1]
                    pj = holder.pop((tname, ec))
                    xt = xt_tiles[(tname, w, 1)]
                    for dc in range(4):
                        nc.tensor.matmul(
                            pj, wsb[:, dc, ec * 128:(ec + 1) * 128],
                            xt[:, dc, :], start=False, stop=(dc == 3))
                    if tname == "q":
                        qt[(w, ec)] = persist.tile(
                            [128, W], BF16, tag=f"qt{w}_{ec}", name=f"qt{w}_{ec}")
                        nc.vector.tensor_copy(qt[(w, ec)], pj)
                    else:
                        nc.vector.tensor_copy(
                            kt_sb[ec][:, w * W:(w + 1) * W], pj)

                def v_first(sc, w=w):
                    pv = ps.tile([128, E_LOCAL], F32, tag="work", bufs=2,
                                 name=f"pv{w}_{sc}")
                    holder[("v", sc)] = pv
                    xt = xt_tiles[("v", w, 0)]
                    for dc in range(4):
                        nc.tensor.matmul(
                            pv, xt[:, dc, sc * 128:(sc + 1) * 128],
                            wv_lo[:, dc, :], start=(dc == 0), stop=False)

                def v_second(sc, w=w):
                    pv = holder.pop(("v", sc))
                    xt = xt_tiles[("v", w, 1)]
                    for dc in range(4):
                        nc.tensor.matmul(
                            pv, xt[:, dc, sc * 128:(sc + 1) * 128],
                            wv_hi[:, dc, :], start=False, stop=(dc == 3))
                    ci = w * 4 + sc
                    nc.vector.tensor_copy(
                        v_sb[ci][:, :, 0:64],
                        pv.rearrange("p (h e) -> p h e", h=H_LOCAL))
                    ones_b = bass.AP(
                        tensor=ones_col.tensor, offset=ones_col.offset,
                        ap=[ones_col.ap[0], [0, H_LOCAL], ones_col.ap[1]])
                    nc.vector.tensor_copy(v_sb[ci][:, :, 64:65], ones_b)

                qi, ki, vi = [], [], []
                for ec in range(NEC):
                    qi.append(lambda ec=ec: qk_first("q", ec))
                    qi.append(lambda ec=ec: qk_second("q", ec))
                for ec in range(NEC):
                    ki.append(lambda ec=ec: qk_first("k", ec))
                    ki.append(lambda ec=ec: qk_second("k", ec))
                for sc in range(4):
                    vi.append(lambda sc=sc: v_first(sc))
                    vi.append(lambda sc=sc: v_second(sc))
                return items, qi, ki, vi

            def out_items(w, tag="work"):
                """Out-projection of window w (needs ctx[(w, *)])."""
                items = []

                def emit(qc, nh):
                    po = ps.tile([128, 512], F32, tag=tag, bufs=2,
                                 name=f"po{qc}_{nh}")
                    for dvc in range(NEC):
                        nc.tensor.matmul(
                            po,
                            ctx[(w, dvc)][:, (qc % 4) * 128:
                                          (qc % 4 + 1) * 128],
                            wo_sb[:, dvc, nh * 512:(nh + 1) * 512],
                            start=(dvc == 0), stop=(dvc == NEC - 1))
                    osb = sm.tile([128, 512], BF16, tag="osb", bufs=2,
                                  name=f"osb{qc}_{nh}")
                    nc.vector.tensor_copy(osb, po)
                    nc.sync.dma_start(
                        out=out[qc * 128:(qc + 1) * 128,
                                nh * 512:(nh + 1) * 512],
                        in_=osb)

                for qc in range(4 * w, 4 * w + 4):
                    for nh in range(2):
                        items.append(lambda qc=qc, nh=nh: emit(qc, nh))
                return items

            def attention_unit(j, hp, tick):
                q0 = j * W
                nlast = 4 * j + 3
                qtile = qt[(j, hp)]
                cpx = [ps.tile([65, W], F32, tag="cpx", bufs=2,
                               name=f"cpx{j}_{hp}_{hi}") for hi in range(2)]
                ctx[(j, hp)] = persist.tile([128, W], BF16, tag=f"ctx{j}_{hp}",
                                            name=f"ctx{j}_{hp}")
                def emit_lg(c):
                    vo = max(0, c * 128 - q0)
                    lg = ps.tile([128, 2 * W], F32, tag="lg", bufs=2,
                                 name=f"lg{j}_{hp}_{c}")
                    pt = sm.tile([128, 2 * W], BF16, tag="pt", bufs=4,
                                 name=f"pt{j}_{hp}_{c}")
                    for hi in range(2):
                        nc.tensor.matmul(
                            lg[:, hi * W + vo:(hi + 1) * W],
                            kt_sb[hp][hi * 64:(hi + 1) * 64,
                                      c * 128:(c + 1) * 128],
                            qtile[hi * 64:(hi + 1) * 64, vo:W],
                            start=True, stop=True)
                    return vo, lg, pt

                def emit_exp(c, vo, lg, pt):
                    if vo >= 256:
                        # separate calls per head skip the vo-wide stale
                        # span between the two heads' column ranges
                        nc.scalar.activation(pt[:, vo:W], lg[:, vo:W], Exp)
                        nc.scalar.activation(pt[:, W + vo:2 * W],
                                             lg[:, W + vo:2 * W], Exp)
                    else:
                        nc.scalar.activation(pt[:, vo:2 * W],
                                             lg[:, vo:2 * W], Exp)
                    if c >= 4 * j:
                        # zero the exp'd upper triangle of the diagonal
                        # 128-block of each head (replaces the -1e9 mask)
                        blk = pt.rearrange("p (h q) -> p h q", h=2)[
                            :, :, vo:vo + 128]
                        nc.gpsimd.affine_select(
                            out=blk, in_=blk,
                            compare_op=mybir.AluOpType.is_ge, fill=0.0,
                            base=0, pattern=[[0, 2], [1, 128]],
                            channel_multiplier=-1)

                def emit_pv(c, vo, pt):
                    for hi in range(2):
                        nc.tensor.matmul(
                            cpx[hi][:, vo:W],
                            v_sb[c][:, hp * 2 + hi, :],
                            pt[:, hi * W + vo:(hi + 1) * W],
                            start=(c == 0), stop=(c == nlast))

                for c in range(4 * j + 4):
                    vo, lg, pt = emit_lg(c)
                    emit_exp(c, vo, lg, pt)
                    if DEBUG and j == 0 and hp == 0 and c == 0:
                        dbg_pt_sb = sm.tile([128, 1024], F32, tag="dbgpt",
                                            bufs=1)
                        nc.vector.tensor_copy(dbg_pt_sb, pt)
                        nc.sync.dma_start(out=dbg_pt[:, :], in_=dbg_pt_sb)
                    emit_pv(c, vo, pt)
                    tick()
                if DEBUG and j == 0 and hp == 0:
                    for hi in range(2):
                        dsb = sm.tile([1, 512], F32, tag=f"dbgden{hi}",
                                      bufs=1, name=f"dbgden{hi}")
                        nc.vector.tensor_copy(dsb, cpx[hi][64:65, :])
                        nc.sync.dma_start(out=dbg_den[hi:hi + 1, :], in_=dsb)
                    csb = sm.tile([64, 512], F32, tag="dbgcpx", bufs=1)
                    nc.vector.tensor_copy(csb, cpx[0][0:64, :])
                    nc.sync.dma_start(out=dbg_cpx[:, :], in_=csb)
                for hi in range(2):
                    bc = sm.tile([64, W], F32, tag="bc", bufs=2,
                                 name=f"bc{j}_{hp}_{hi}")
                    # NB: reciprocal_approx_fast is a custom-DVE op that
                    # drops the input AP's base partition, so the PSUM
                    # denominator row must be copied to partition 0 first.
                    nc.vector.tensor_copy(bc[0:1, :], cpx[hi][64:65, :])
                    nc.vector.reciprocal_approx_fast(
                        out=bc[0:1, :], in_=bc[0:1, :])
                    nc.gpsimd.partition_broadcast(bc, bc[0:1, :])
                    if DEBUG and j == 0 and hp == 0:
                        bsb = sm.tile([64, 512], F32, tag=f"dbgbc{hi}",
                                      bufs=1, name=f"dbgbc{hi}")
                        nc.vector.tensor_copy(bsb, bc)
                        nc.sync.dma_start(out=dbg_bc[hi, :, :], in_=bsb)
                    nc.vector.tensor_mul(
                        ctx[(j, hp)][hi * 64:(hi + 1) * 64, :],
                        cpx[hi][0:64, :], bc)

            # ---- schedule ----
            p0d, p0q, p0k, p0v = proj_items(0)
            p1d, p1q, p1k, p1v = proj_items(1)
            p2d, p2q, p2k, p2v = proj_items(2)
            p3d, p3q, p3k, p3v = proj_items(3)

            # prologue: just enough for attention(0,0) chunk 0
            for it in p0q[0:2] + p0k[0:2] + p0v[0:2]:
                it()
            if DEBUG:
                t1 = sm.tile([128, 512], F32, tag="dbg1", bufs=1)
                nc.vector.tensor_copy(t1, qt[(0, 0)])
                nc.sync.dma_start(out=dbg_qt[:, :], in_=t1)
                t2 = sm.tile([128, 512], F32, tag="dbg2", bufs=1)
                nc.vector.tensor_copy(t2, kt_sb[0][:, 0:512])
                nc.sync.dma_start(out=dbg_kt[:, :], in_=t2)
                t3 = sm.tile([128, 8, 65], F32, tag="dbg3", bufs=1)
                nc.vector.tensor_copy(t3, v_sb[0])
                nc.sync.dma_start(out=dbg_v[:, :, :], in_=t3)

            phase_fill = {
                0: (p0v[2:4] + p0v[4:6] + p0v[6:8]
                    + p0q[2:4] + p0k[2:4] + p0q[4:6] + p0k[4:6]
                    + p0q[6:8] + p0k[6:8] + p1q + p1k),
                1: p1v + p2d + p2q + p2k + p2v,
                2: p3d + p3q + p3k + p3v + out_items(0),
                3: out_items(1) + out_items(2),
            }
            for j in range(NW):
                items = phase_fill[j]
                nchunks = (4 * j + 4) * NEC
                state = {"i": 0, "t": 0}

                def tick(items=items, nchunks=nchunks, state=state):
                    state["t"] += 1
                    target = min(len(items),
                                 len(items) * state["t"] // nchunks + 2)
                    while state["i"] < target:
                        items[state["i"]]()
                        state["i"] += 1

                for hp in range(NEC):
                    attention_unit(j, hp, tick)
                while state["i"] < len(items):
                    items[state["i"]]()
                    state["i"] += 1
            # after the last exp the lg banks are idle; running the final
            # out-projection in them doubles the groups in flight
            for it in out_items(3, tag="lg"):
                it()
            if DEBUG:
                t4 = sm.tile([128, 512], F32, tag="dbg4", bufs=1)
                nc.vector.tensor_copy(t4, ctx[(0, 0)])
                nc.sync.dma_start(out=dbg_ctx[:, :], in_=t4)

    nc.compile()
    return nc


def _in_maps(queries, keys, values, Wq, Wk, Wv, Wo):
    import ml_dtypes

    bf16 = ml_dtypes.bfloat16
    scale = np.float32(0.125)  # (DK//H) ** -0.5, exact power of two
    xts = []
    for b in range(B):
        xts.append({
            "xqt": np.ascontiguousarray(
                np.asarray(queries[b], np.float32).T).astype(bf16),
            "xkt": np.ascontiguousarray(
                np.asarray(keys[b], np.float32).T).astype(bf16),
            "xvt": np.ascontiguousarray(
                np.asarray(values[b], np.float32).T).astype(bf16),
        })
    wslices = []
    for g in range(2):
        sl = slice(g * E_LOCAL, (g + 1) * E_LOCAL)
        wslices.append({
            "wq": np.ascontiguousarray(
                np.asarray(Wq[:, sl], np.float32) * scale).astype(bf16),
            "wk": np.ascontiguousarray(
                np.asarray(Wk[:, sl], np.float32)).astype(bf16),
            "wv": np.ascontiguousarray(
                np.asarray(Wv[:, sl], np.float32)).astype(bf16),
            "wo": np.ascontiguousarray(
                np.asarray(Wo[sl, :], np.float32)).astype(bf16),
        })
    in_maps = []
    for c in range(N_CORES):
        b, g = divmod(c, 2)
        m = dict(xts[b])
        m.update(wslices[g])
        in_maps.append(m)
    return in_maps


def kernel(queries, keys, values, mask=None, Wq=None, Wk=None, Wv=None,
           Wo=None, **_ignored):
    from concourse.bass_utils import run_bass_kernel_spmd

    if "nc" not in _cached:
        _cached["nc"] = _build()
    nc = _cached["nc"]

    in_maps = _in_maps(queries, keys, values, Wq, Wk, Wv, Wo)
    res = run_bass_kernel_spmd(nc, in_maps, core_ids=list(range(N_CORES)))
    outs = res.results
    full = np.empty((B, S, HID), np.float32)
    for b in range(B):
        full[b] = (outs[2 * b]["out"].astype(np.float32)
                   + outs[2 * b + 1]["out"].astype(np.float32))
    return full


def run_traced(inputs, tmpdir=None):
    """Run once with NTFF tracing; returns BassKernelResults."""
    from concourse.bass_utils import run_bass_kernel_spmd

    if "nc" not in _cached:
        _cached["nc"] = _build()
    nc = _cached["nc"]
    in_maps = _in_maps(inputs["queries"], inputs["keys"], inputs["values"],
                       inputs["Wq"], inputs["Wk"], inputs["Wv"], inputs["Wo"])
    return run_bass_kernel_spmd(nc, in_maps, core_ids=list(range(N_CORES)),
                                trace=True, tmpdir=tmpdir)


# revision 26
# speedup vs baseline: 1.0104x; 1.0104x over previous
"""Multi-head causal attention kernel for Trainium2 (8 NeuronCores).

Problem: B=4, S=2048, HID=1024, H=16 heads (head_dim 64), causal mask,
fp32 I/O.  out = softmax(mask + (XqWq)(XkWk)^T/8) (XvWv) Wo

Sharding: 8 cores = 4 batches x 2 head-groups.  Core c handles batch
c//2 and heads (c%2)*8 .. +8 (dk slice of 512).  Each core computes a
full-shape [S, HID] partial output (its head-group's contribution
through Wo); the host sums the two partials per batch.

v2 design (vs the 495us baseline):
  - X is transposed to [d, s] and cast to bf16 on the HOST, so the
    kernel needs no PE-transposes and no PSUM->SBUF transpose
    evacuations (the baseline burned ~107us of DVE and ~20us of PE on
    those).  All matmul operands are bf16 (same PE rate, half the DMA
    and SBUF traffic, 2x DVE modes on evacuations).
  - Attention runs in transposed [k, q] orientation per 512-wide
    q-window and head-pair: logits^T = kT-chunk x qT, exp on ScalarE
    (PSUM->SBUF, bf16 out), causal masking done by ZEROING the exp'd
    upper-triangle of the diagonal blocks on the otherwise-idle GpSimd
    engine (affine_select), PV accumulates ctx^T in PSUM with a ones
    column in v producing softmax denominators for free.
  - Normalization: DVE copy of the PSUM denominator row to partition 0
    (reciprocal_approx_fast drops input base-partition offsets), fast
    reciprocal, GpSimd partition-broadcast, one DVE multiply-evacuate.
  - Projection/out-projection matmuls are emitted in ~4-MM "filler"
    items interleaved between attention chunks so the PE stays dense
    (HAM stays at full clock) while ScalarE grinds exps.
"""

import numpy as np

B, S, HID = 4, 2048, 1024
H_LOCAL, E_LOCAL = 8, 512  # heads / dk columns handled per core
N_CORES = 8

_cached = {}
DEBUG = False


def _build():
    from concourse import bacc
    import concourse.bass as bass
    import concourse.mybir as mybir
    import concourse.tile as tile

    F32 = mybir.dt.float32
    BF16 = mybir.dt.bfloat16
    Exp = mybir.ActivationFunctionType.Exp

    NDC = HID // 128   # 8 d-chunks
    NEC = E_LOCAL // 128  # 4 e-chunks = head pairs
    NKC = S // 128     # 16 k-chunks
    W = 512            # q-window
    NW = S // W        # 4 windows

    nc = bacc.Bacc()
    xqt = nc.dram_tensor("xqt", [HID, S], BF16, kind="ExternalInput")
    xkt = nc.dram_tensor("xkt", [HID, S], BF16, kind="ExternalInput")
    xvt = nc.dram_tensor("xvt", [HID, S], BF16, kind="ExternalInput")
    wq = nc.dram_tensor("wq", [HID, E_LOCAL], BF16, kind="ExternalInput")
    wk = nc.dram_tensor("wk", [HID, E_LOCAL], BF16, kind="ExternalInput")
    wv = nc.dram_tensor("wv", [HID, E_LOCAL], BF16, kind="ExternalInput")
    wo = nc.dram_tensor("wo", [E_LOCAL, HID], BF16, kind="ExternalInput")
    out = nc.dram_tensor("out", [S, HID], BF16, kind="ExternalOutput")
    if DEBUG:
        dbg_qt = nc.dram_tensor("dbg_qt", [128, 512], F32,
                                kind="ExternalOutput")
        dbg_kt = nc.dram_tensor("dbg_kt", [128, 512], F32,
                                kind="ExternalOutput")
        dbg_v = nc.dram_tensor("dbg_v", [128, 8, 65], F32,
                               kind="ExternalOutput")
        dbg_den = nc.dram_tensor("dbg_den", [2, 512], F32,
                                 kind="ExternalOutput")
        dbg_cpx = nc.dram_tensor("dbg_cpx", [64, 512], F32,
                                 kind="ExternalOutput")
        dbg_ctx = nc.dram_tensor("dbg_ctx", [128, 512], F32,
                                 kind="ExternalOutput")
        dbg_pt = nc.dram_tensor("dbg_pt", [128, 1024], F32,
                                kind="ExternalOutput")
        dbg_bc = nc.dram_tensor("dbg_bc", [2, 64, 512], F32,
                                kind="ExternalOutput")

    with tile.TileContext(nc) as tc:
        with (
            tc.sbuf_pool(name="consts", bufs=1) as consts,
            tc.sbuf_pool(name="persist", bufs=1) as persist,
            tc.sbuf_pool(name="sm", bufs=1) as sm,
            tc.psum_pool(name="ps", bufs=1) as ps,
        ):
            ones_col = consts.tile([128, 1], BF16)
            nc.vector.memset(ones_col, 1.0)

            HDC = NDC // 2
            wq_lo = persist.tile([128, HDC, E_LOCAL], BF16, tag="wql")
            wq_hi = persist.tile([128, HDC, E_LOCAL], BF16, tag="wqh")
            wk_lo = persist.tile([128, HDC, E_LOCAL], BF16, tag="wkl")
            wk_hi = persist.tile([128, HDC, E_LOCAL], BF16, tag="wkh")
            wv_lo = persist.tile([128, HDC, E_LOCAL], BF16, tag="wvl")
            wv_hi = persist.tile([128, HDC, E_LOCAL], BF16, tag="wvh")
            wo_sb = persist.tile([128, NEC, HID], BF16, tag="wo")
            wpart = {"q": (wq_lo, wq_hi), "k": (wk_lo, wk_hi),
                     "v": (wv_lo, wv_hi)}

            kt_sb = [persist.tile([128, S], BF16, tag=f"kt{i}", name=f"kt{i}")
                     for i in range(NEC)]
            v_sb = [persist.tile([128, H_LOCAL, 65], BF16, tag=f"v{i}", name=f"v{i}")
                    for i in range(NKC)]
            qt = {}   # (w, ec) -> [128, W] bf16
            ctx = {}  # (w, hp) -> [128, W] bf16

            xdram = {"q": xqt, "k": xkt, "v": xvt}
            xt_tiles = {}

            def load_x(tname, w, half):
                t = sm.tile([128, HDC, W], BF16, tag=f"x{tname}{half}",
                            bufs=2, name=f"x{tname}_{w}_{half}")
                nc.sync.dma_start(
                    out=t,
                    in_=xdram[tname][half * 512:(half + 1) * 512,
                                     w * W:(w + 1) * W].rearrange(
                        "(dc p) s -> p dc s", p=128))
                xt_tiles[(tname, w, half)] = t

            # weights + first two x-windows; ordering puts the tensors
            # needed first at the head of the DMA queues.
            def wdma(dram, lo, hi):
                nc.sync.dma_start(
                    out=lo, in_=dram[0:512, :].rearrange(
                        "(dc p) e -> p dc e", p=128))
                return lambda: nc.sync.dma_start(
                    out=hi, in_=dram[512:1024, :].rearrange(
                        "(dc p) e -> p dc e", p=128))

            hq = wdma(wq, wq_lo, wq_hi)
            load_x("q", 0, 0)
            hq()
            load_x("q", 0, 1)
            hk = wdma(wk, wk_lo, wk_hi)
            load_x("k", 0, 0)
            hk()
            load_x("k", 0, 1)
            hv = wdma(wv, wv_lo, wv_hi)
            load_x("v", 0, 0)
            hv()
            load_x("v", 0, 1)
            for t in ("q", "k", "v"):
                load_x(t, 1, 0)
                load_x(t, 1, 1)
            nc.sync.dma_start(
                out=wo_sb, in_=wo.rearrange("(dv p) n -> p dv n", p=128))

            def proj_items(w):
                """Emission closures (~4 matmuls each) projecting window w."""
                items = []
                if w >= 2:
                    def dma_item(w=w):
                        for t in ("q", "k", "v"):
                            load_x(t, w, 0)
                            load_x(t, w, 1)
                    items.append(dma_item)

                holder = {}

                def qk_first(tname, ec, w=w):
                    wsb = wpart[tname][0]
                    pj = ps.tile([128, W], F32, tag="work", bufs=2,
                                 name=f"pj{tname}{w}_{ec}")
                    holder[(tname, ec)] = pj
                    xt = xt_tiles[(tname, w, 0)]
                    for dc in range(4):
                        nc.tensor.matmul(
                            pj, wsb[:, dc, ec * 128:(ec + 1) * 128],
                            xt[:, dc, :], start=(dc == 0), stop=False)

                def qk_second(tname, ec, w=w):
                    wsb = wpart[tname][1]
                    pj = holder.pop((tname, ec))
                    xt = xt_tiles[(tname, w, 1)]
                    for dc in range(4):
                        nc.tensor.matmul(
                            pj, wsb[:, dc, ec * 128:(ec + 1) * 128],
                            xt[:, dc, :], start=False, stop=(dc == 3))
                    if tname == "q":
                        qt[(w, ec)] = persist.tile(
                            [128, W], BF16, tag=f"qt{w}_{ec}", name=f"qt{w}_{ec}")
                        nc.vector.tensor_copy(qt[(w, ec)], pj)
                    else:
                        nc.vector.tensor_copy(
                            kt_sb[ec][:, w * W:(w + 1) * W], pj)

                def v_first(sc, w=w):
                    pv = ps.tile([128, E_LOCAL], F32, tag="work", bufs=2,
                                 name=f"pv{w}_{sc}")
                    holder[("v", sc)] = pv
                    xt = xt_tiles[("v", w, 0)]
                    for dc in range(4):
                        nc.tensor.matmul(
                            pv, xt[:, dc, sc * 128:(sc + 1) * 128],
                            wv_lo[:, dc, :], start=(dc == 0), stop=False)

                def v_second(sc, w=w):
                    pv = holder.pop(("v", sc))
                    xt = xt_tiles[("v", w, 1)]
                    for dc in range(4):
                        nc.tensor.matmul(
                            pv, xt[:, dc, sc * 128:(sc + 1) * 128],
                            wv_hi[:, dc, :], start=False, stop=(dc == 3))
                    ci = w * 4 + sc
                    nc.vector.tensor_copy(
                        v_sb[ci][:, :, 0:64],
                        pv.rearrange("p (h e) -> p h e", h=H_LOCAL))
                    ones_b = bass.AP(
                        tensor=ones_col.tensor, offset=ones_col.offset,
                        ap=[ones_col.ap[0], [0, H_LOCAL], ones_col.ap[1]])
                    nc.vector.tensor_copy(v_sb[ci][:, :, 64:65], ones_b)

                qi, ki, vi = [], [], []
                for ec in range(NEC):
                    qi.append(lambda ec=ec: qk_first("q", ec))
                    qi.append(lambda ec=ec: qk_second("q", ec))
                for ec in range(NEC):
                    ki.append(lambda ec=ec: qk_first("k", ec))
                    ki.append(lambda ec=ec: qk_second("k", ec))
                for sc in range(4):
                    vi.append(lambda sc=sc: v_first(sc))
                    vi.append(lambda sc=sc: v_second(sc))
                return items, qi, ki, vi

            def out_items(w, tag="work"):
                """Out-projection of window w (needs ctx[(w, *)])."""
                items = []

                def emit(qc, nh):
                    po = ps.tile([128, 512], F32, tag=tag, bufs=2,
                                 name=f"po{qc}_{nh}")
                    for dvc in range(NEC):
                        nc.tensor.matmul(
                            po,
                            ctx[(w, dvc)][:, (qc % 4) * 128:
                                          (qc % 4 + 1) * 128],
                            wo_sb[:, dvc, nh * 512:(nh + 1) * 512],
                            start=(dvc == 0), stop=(dvc == NEC - 1))
                    osb = sm.tile([128, 512], BF16, tag="osb", bufs=2,
                                  name=f"osb{qc}_{nh}")
                    nc.vector.tensor_copy(osb, po)
                    nc.sync.dma_start(
                        out=out[qc * 128:(qc + 1) * 128,
                                nh * 512:(nh + 1) * 512],
                        in_=osb)

                for qc in range(4 * w, 4 * w + 4):
                    for nh in range(2):
                        items.append(lambda qc=qc, nh=nh: emit(qc, nh))
                return items

            def attention_unit(j, hp, tick):
                q0 = j * W
                nlast = 4 * j + 3
                qtile = qt[(j, hp)]
                cpx = [ps.tile([65, W], F32, tag="cpx", bufs=2,
                               name=f"cpx{j}_{hp}_{hi}") for hi in range(2)]
                ctx[(j, hp)] = persist.tile([128, W], BF16, tag=f"ctx{j}_{hp}",
                                            name=f"ctx{j}_{hp}")
                def emit_lg(c):
                    vo = max(0, c * 128 - q0)
                    lg = ps.tile([128, 2 * W], F32, tag="lg", bufs=2,
                                 name=f"lg{j}_{hp}_{c}")
                    pt = sm.tile([128, 2 * W], BF16, tag="pt", bufs=4,
                                 name=f"pt{j}_{hp}_{c}")
                    for hi in range(2):
                        nc.tensor.matmul(
                            lg[:, hi * W + vo:(hi + 1) * W],
                            kt_sb[hp][hi * 64:(hi + 1) * 64,
                                      c * 128:(c + 1) * 128],
                            qtile[hi * 64:(hi + 1) * 64, vo:W],
                            start=True, stop=True)
                    return vo, lg, pt

                def emit_exp(c, vo, lg, pt):
                    if vo >= 256:
                        # separate calls per head skip the vo-wide stale
                        # span between the two heads' column ranges
                        nc.scalar.activation(pt[:, vo:W], lg[:, vo:W], Exp)
                        nc.scalar.activation(pt[:, W + vo:2 * W],
                                             lg[:, W + vo:2 * W], Exp)
                    else:
                        nc.scalar.activation(pt[:, vo:2 * W],
                                             lg[:, vo:2 * W], Exp)
                    if c >= 4 * j:
                        # zero the exp'd upper triangle of the diagonal
                        # 128-block of each head (replaces the -1e9 mask)
                        blk = pt.rearrange("p (h q) -> p h q", h=2)[
                            :, :, vo:vo + 128]
                        nc.gpsimd.affine_select(
                            out=blk, in_=blk,
                            compare_op=mybir.AluOpType.is_ge, fill=0.0,
                            base=0, pattern=[[0, 2], [1, 128]],
                            channel_multiplier=-1)

                def emit_pv(c, vo, pt):
                    for hi in range(2):
                        nc.tensor.matmul(
                            cpx[hi][:, vo:W],
                            v_sb[c][:, hp * 2 + hi, :],
                            pt[:, hi * W + vo:(hi + 1) * W],
                            start=(c == 0), stop=(c == nlast))

                for c in range(4 * j + 4):
                    vo, lg, pt = emit_lg(c)
                    emit_exp(c, vo, lg, pt)
                    if DEBUG and j == 0 and hp == 0 and c == 0:
                        dbg_pt_sb = sm.tile([128, 1024], F32, tag="dbgpt",
                                            bufs=1)
                        nc.vector.tensor_copy(dbg_pt_sb, pt)
                        nc.sync.dma_start(out=dbg_pt[:, :], in_=dbg_pt_sb)
                    emit_pv(c, vo, pt)
                    tick()
                if DEBUG and j == 0 and hp == 0:
                    for hi in range(2):
                        dsb = sm.tile([1, 512], F32, tag=f"dbgden{hi}",
                                      bufs=1, name=f"dbgden{hi}")
                        nc.vector.tensor_copy(dsb, cpx[hi][64:65, :])
                        nc.sync.dma_start(out=dbg_den[hi:hi + 1, :], in_=dsb)
                    csb = sm.tile([64, 512], F32, tag="dbgcpx", bufs=1)
                    nc.vector.tensor_copy(csb, cpx[0][0:64, :])
                    nc.sync.dma_start(out=dbg_cpx[:, :], in_=csb)
                for hi in range(2):
                    bc = sm.tile([64, W], F32, tag="bc", bufs=2,
                                 name=f"bc{j}_{hp}_{hi}")
                    # NB: reciprocal_approx_fast is a custom-DVE op that
                    # drops the input AP's base partition, so the PSUM
                    # denominator row must be copied to partition 0 first.
                    nc.vector.tensor_copy(bc[0:1, :], cpx[hi][64:65, :])
                    nc.vector.reciprocal_approx_fast(
                        out=bc[0:1, :], in_=bc[0:1, :])
                    nc.gpsimd.partition_broadcast(bc, bc[0:1, :])
                    if DEBUG and j == 0 and hp == 0:
                        bsb = sm.tile([64, 512], F32, tag=f"dbgbc{hi}",
                                      bufs=1, name=f"dbgbc{hi}")
                        nc.vector.tensor_copy(bsb, bc)
                        nc.sync.dma_start(out=dbg_bc[hi, :, :], in_=bsb)
                    nc.vector.tensor_mul(
                        ctx[(j, hp)][hi * 64:(hi + 1) * 64, :],
                        cpx[hi][0:64, :], bc)

            # ---- schedule ----
            p0d, p0q, p0k, p0v = proj_items(0)
            p1d, p1q, p1k, p1v = proj_items(1)
            p2d, p2q, p2k, p2v = proj_items(2)
            p3d, p3q, p3k, p3v = proj_items(3)

            # prologue: just enough for attention(0,0) chunk 0
            for it in p0q[0:2] + p0k[0:2] + p0v[0:2]:
                it()
            if DEBUG:
                t1 = sm.tile([128, 512], F32, tag="dbg1", bufs=1)
                nc.vector.tensor_copy(t1, qt[(0, 0)])
                nc.sync.dma_start(out=dbg_qt[:, :], in_=t1)
                t2 = sm.tile([128, 512], F32, tag="dbg2", bufs=1)
                nc.vector.tensor_copy(t2, kt_sb[0][:, 0:512])
                nc.sync.dma_start(out=dbg_kt[:, :], in_=t2)
                t3 = sm.tile([128, 8, 65], F32, tag="dbg3", bufs=1)
                nc.vector.tensor_copy(t3, v_sb[0])
                nc.sync.dma_start(out=dbg_v[:, :, :], in_=t3)

            phase_fill = {
                0: (p0v[2:4] + p0v[4:6] + p0v[6:8]
                    + p0q[2:4] + p0k[2:4] + p0q[4:6] + p0k[4:6]
                    + p0q[6:8] + p0k[6:8] + p1q + p1k),
                1: p1v + p2d + p2q + p2k + p2v,
                2: p3d + p3q + p3k + p3v + out_items(0),
                3: out_items(1) + out_items(2),
            }
            for j in range(NW):
                items = phase_fill[j]
                nchunks = (4 * j + 4) * NEC
                state = {"i": 0, "t": 0}

                def tick(items=items, nchunks=nchunks, state=state):
                    state["t"] += 1
                    target = min(len(items),
                                 len(items) * state["t"] // nchunks + 2)
                    while state["i"] < target:
                        items[state["i"]]()
                        state["i"] += 1

                for hp in range(NEC):
                    attention_unit(j, hp, tick)
                while state["i"] < len(items):
                    items[state["i"]]()
                    state["i"] += 1
            # after the last exp the lg banks are idle; running the final
            # out-projection in them doubles the groups in flight
            for it in out_items(3, tag="lg"):
                it()
            if DEBUG:
                t4 = sm.tile([128, 512], F32, tag="dbg4", bufs=1)
                nc.vector.tensor_copy(t4, ctx[(0, 0)])
                nc.sync.dma_start(out=dbg_ctx[:, :], in_=t4)

    nc.compile()
    return nc


def _in_maps(queries, keys, values, Wq, Wk, Wv, Wo):
    import ml_dtypes

    bf16 = ml_dtypes.bfloat16
    scale = np.float32(0.125)  # (DK//H) ** -0.5, exact power of two
    xts = []
    for b in range(B):
        xts.append({
            "xqt": np.ascontiguousarray(
                np.asarray(queries[b], np.float32).T).astype(bf16),
            "xkt": np.ascontiguousarray(
                np.asarray(keys[b], np.float32).T).astype(bf16),
            "xvt": np.ascontiguousarray(
                np.asarray(values[b], np.float32).T).astype(bf16),
        })
    wslices = []
    for g in range(2):
        sl = slice(g * E_LOCAL, (g + 1) * E_LOCAL)
        wslices.append({
            "wq": np.ascontiguousarray(
                np.asarray(Wq[:, sl], np.float32) * scale).astype(bf16),
            "wk": np.ascontiguousarray(
                np.asarray(Wk[:, sl], np.float32)).astype(bf16),
            "wv": np.ascontiguousarray(
                np.asarray(Wv[:, sl], np.float32)).astype(bf16),
            "wo": np.ascontiguousarray(
                np.asarray(Wo[sl, :], np.float32)).astype(bf16),
        })
    in_maps = []
    for c in range(N_CORES):
        b, g = divmod(c, 2)
        m = dict(xts[b])
        m.update(wslices[g])
        in_maps.append(m)
    return in_maps


def kernel(queries, keys, values, mask=None, Wq=None, Wk=None, Wv=None,
           Wo=None, **_ignored):
    from concourse.bass_utils import run_bass_kernel_spmd

    if "nc" not in _cached:
        _cached["nc"] = _build()
    nc = _cached["nc"]

    in_maps = _in_maps(queries, keys, values, Wq, Wk, Wv, Wo)
    res = run_bass_kernel_spmd(nc, in_maps, core_ids=list(range(N_CORES)))
    outs = res.results
    full = np.empty((B, S, HID), np.float32)
    for b in range(B):
        full[b] = (outs[2 * b]["out"].astype(np.float32)
                   + outs[2 * b + 1]["out"].astype(np.float32))
    return full


def run_traced(inputs, tmpdir=None):
    """Run once with NTFF tracing; returns BassKernelResults."""
    from concourse.bass_utils import run_bass_kernel_spmd

    if "nc" not in _cached:
        _cached["nc"] = _build()
    nc = _cached["nc"]
    in_maps = _in_maps(inputs["queries"], inputs["keys"], inputs["values"],
                       inputs["Wq"], inputs["Wk"], inputs["Wv"], inputs["Wo"])
    return run_bass_kernel_spmd(nc, in_maps, core_ids=list(range(N_CORES)),
                                trace=True, tmpdir=tmpdir)
